# revision 8
# baseline (speedup 1.0000x reference)
"""Multi-head causal attention (B=2, S=2048, D=1024, H=16) on 8 TRN2 NeuronCores.

Sharding: batch x head-group.  Core i handles batch b = i//4 and head-group
hg = i%4 (4 heads = 256 projection columns).  Each core computes
  Q^T/K^T/V = proj(X_b) for its 256 columns, causal attention for its 4
  heads, and a partial output  ctx_slice @ Wo[256-row slice]  ->
  [2048, 1024] fp16 partial.  Host sums the 4 partials per batch (fp32) and
  adds bo (standard tensor-parallel row-sharded out-projection unshard).

v2 structure (vs the 217us baseline):
  - PE warm-up burst at t=0 + junk-MM sprinkles inside the transpose
    chains: PE-transposes don't count as "busy" for the HAM clock gate, so
    without real matmuls in the window the whole prologue runs at 1.2GHz.
  - X is loaded with 8 DMAs (2 token-tiles each) alternating sync/scalar
    HWDGE rings; weights ride the gpsimd SWDGE ring (V first).
  - Engine-queue ordering matters (all queues are FIFO): casts precede
    weight converts on DVE, Wo's convert is emitted only before outproj,
    and softmax normalization (recip/broadcast/mul) is deferred one head
    so gpsimd's partition_broadcast never blocks the next head's masks.
  - qh-major attention with pure-PE blocks (late X transposes + V/QK
    projections, out-projection of finished q-rows) between heads.
  - causal masks + partial-chunk zeroing on gpsimd (frees DVE).
  - fp16 DRAM output (halves output DMA).
PSUM: tag "work" [128,1024]f32 x2 + tag "ctp" x2 = 8 banks exactly.
"""

import numpy as np

import concourse.bass as bass
import concourse.mybir as mybir
import concourse.tile as tile
from concourse import bacc
from concourse.bass_utils import run_bass_kernel_spmd
from concourse.masks import make_identity

F32 = mybir.dt.float32
F32R = mybir.dt.float32r
F16 = mybir.dt.float16
BF16 = mybir.dt.bfloat16
AF = mybir.ActivationFunctionType

B, S, D = 2, 2048, 1024
H, HD = 16, 64
NCORES = 8
CG = 256            # projection columns per core (4 heads)
TOK_TILES = S // 128   # 16
D_CHUNKS = D // 128    # 8
QH = 2              # q halves
QHW = 1024          # q-half width
VW = 128            # [1 | pad | V(64)] per head in vt; V at 64:128 (base-64 slice)


def _build_program():
    nc = bacc.Bacc("TRN2", target_bir_lowering=False, debug=False)

    x_d = nc.dram_tensor("X", [S, D], F32, kind="ExternalInput").ap()
    wq_d = nc.dram_tensor("Wq", [D, CG], F32, kind="ExternalInput").ap()
    wk_d = nc.dram_tensor("Wk", [D, CG], F32, kind="ExternalInput").ap()
    wv_d = nc.dram_tensor("Wv", [D, CG], F32, kind="ExternalInput").ap()
    wo_d = nc.dram_tensor("Wo", [CG, D], F32, kind="ExternalInput").ap()
    out_d = nc.dram_tensor("out", [S, D], F16, kind="ExternalOutput").ap()

    with tile.TileContext(nc) as tc:
        _emit(nc, tc, x_d, wq_d, wk_d, wv_d, wo_d, out_d)
    nc.compile()
    return nc


def _emit(nc, tc, x_d, wq_d, wk_d, wv_d, wo_d, out_d):
    with (
        tc.sbuf_pool(name="persist", bufs=1) as pp,
        tc.sbuf_pool(name="work", bufs=1) as wp,
        tc.psum_pool(name="ps", bufs=1) as pq,
    ):
        # ---- persistent SBUF tensors
        xt = pp.tile([128, D_CHUNKS, S], F16, name="xt")        # X^T  [dval, dchunk, tok]
        qt = pp.tile([128, 2, S], F16, name="qt")               # Q^T  [col, coltile, tok]
        kt = pp.tile([128, 2, S], F16, name="kt")
        vt = pp.tile([128, TOK_TILES, 4, VW], BF16, name="vt")  # [1|pad|V] per (ktile, head)
        ctxT = pp.tile([128, 2, S], BF16, name="ctxT")
        wqh = pp.tile([128, D_CHUNKS, CG], F16, name="wqh")
        wkh = pp.tile([128, D_CHUNKS, CG], F16, name="wkh")
        wvh = pp.tile([128, D_CHUNKS, CG], F16, name="wvh")
        wob = pp.tile([128, 2, D], BF16, name="wob")
        ident = pp.tile([128, 128], F16, name="ident")
        ones_row = pp.tile([1, 128], BF16, name="ones_row")

        # ---- identity first (feeds warm-up + transposes), then PE warm-up:
        # junk matmuls with no data deps keep the PE HAM window busy from
        # ~0us so the projection phase runs at 2.4GHz, not 1.2.
        make_identity(nc, ident)
        nc.gpsimd.memset(ones_row, 1.0)
        pw = pq.tile([128, 1024], F32, tag="ctp", bufs=2, name="pw")
        for i in range(35):
            nc.tensor.matmul(pw[:, 0:128], lhsT=ident, rhs=ident,
                             start=True, stop=True)

        # ---- DMAs: weights on the gpsimd SWDGE ring (V first — needed
        # earliest), X as 8x 2-tile loads alternating sync/scalar HWDGE.
        # weights lead on the sync ring (V then K), X pairs alternate
        # scalar/sync so pair 0 lands first; Wq rides late on scalar, Wo
        # last on sync.  Two HWDGE rings split the 12MB inbound stream.
        wvs = wp.tile([128, D_CHUNKS, CG], F32, tag="wstage", bufs=2, name="wvs")
        nc.sync.dma_start(wvs, wv_d.rearrange("(dc p) c -> p dc c", p=128))
        wks = wp.tile([128, D_CHUNKS, CG], F32, tag="wstage", bufs=2, name="wks")
        nc.sync.dma_start(wks, wk_d.rearrange("(dc p) c -> p dc c", p=128))

        xs_tiles = []
        for xp in range(8):
            xs = wp.tile([128, 2, D], F32, tag="xs", bufs=4, name=f"xs{xp}")
            eng = nc.scalar if xp % 2 == 0 else nc.sync
            eng.dma_start(
                xs, x_d[xp * 256:(xp + 1) * 256, :].rearrange(
                    "(c p) d -> p c d", p=128))
            xs_tiles.append(xs)

        wqs = wp.tile([128, D_CHUNKS, CG], F32, tag="wstage2", bufs=2, name="wqs")
        nc.scalar.dma_start(wqs, wq_d.rearrange("(dc p) c -> p dc c", p=128))
        wos = wp.tile([128, 2, D], F32, tag="wstage2", bufs=2, name="wos")
        nc.sync.dma_start(wos, wo_d.rearrange("(ct p) n -> p ct n", p=128))

        # V ones columns (GPS; right after the SWDGE issues)
        for h in range(4):
            nc.gpsimd.memset(vt[:, :, h, 0:1], 1.0)

        # ---- helpers ------------------------------------------------------
        def xpair_body(xp, xh):
            """transpose + V-projection for token tiles 2xp, 2xp+1.
            Junk matmuls (HAM keep-warm) target the unused bank 1 of the
            V psum tile — transposes alone don't register as PE activity."""
            for tl in range(2):
                t = 2 * xp + tl
                vps = pq.tile([128, 1024], F32, tag="work", bufs=2,
                              name=f"vps{t}")
                for dp in range(2):
                    wk_ps = pq.tile([128, 1024], F32, tag="work", bufs=2,
                                    name=f"xtp{t}_{dp}")
                    xtp = wk_ps[:, 0:256].bitcast(F16)  # [128, 512] f16 view
                    for dd in range(4):
                        d = dp * 4 + dd
                        nc.tensor.transpose(
                            xtp[:, dd * 128:(dd + 1) * 128],
                            xh[:, tl, d * 128:(d + 1) * 128], ident)
                        if dd % 2 == 1:
                            nc.tensor.matmul(vps[:, 512:576], lhsT=ident,
                                             rhs=ident[:, 0:64],
                                             start=True, stop=True)
                    nc.scalar.copy(
                        xt[:, dp * 4:(dp + 1) * 4, t * 128:(t + 1) * 128],
                        xtp.rearrange("p (dd c) -> p dd c", dd=4))
                # V projection for tile t
                for d in range(D_CHUNKS):
                    nc.tensor.matmul(
                        vps[:, 0:CG],
                        lhsT=xt[:, d, t * 128:(t + 1) * 128],
                        rhs=wvh[:, d, :],
                        start=(d == 0), stop=(d == D_CHUNKS - 1))
                nc.vector.tensor_copy(
                    vt[:, t, :, 64:VW],
                    vps[:, 0:CG].rearrange("p (h c) -> p h c", h=4))

        def xpair_chain(xp):
            xh = wp.tile([128, 2, D], F16, tag="xh", bufs=2, name=f"xh{xp}")
            nc.vector.tensor_copy(xh, xs_tiles[xp])
            xpair_body(xp, xh)

        def qk_block(w_sb, dst, ct, t4pair):
            """Q^T or K^T for col-tile ct, tokens [t4pair*1024, +1024).
            d-outer: one stationary load covers two 512-token chunks."""
            ps = pq.tile([128, 1024], F32, tag="work", bufs=2,
                         name=f"qk{ct}_{t4pair}")
            base = t4pair * 1024
            for d in range(D_CHUNKS):
                for ch in range(2):
                    nc.tensor.matmul(
                        ps[:, ch * 512:(ch + 1) * 512],
                        lhsT=w_sb[:, d, ct * 128:(ct + 1) * 128],
                        rhs=xt[:, d, base + ch * 512: base + (ch + 1) * 512],
                        start=(d == 0), stop=(d == D_CHUNKS - 1))
            nc.scalar.copy(dst[:, ct, base:base + 1024], ps)

        pending_norm = []

        def flush_norm():
            while pending_norm:
                pending_norm.pop(0)()

        def attention(h, qh):
            """Causal attention for head h, q-half qh (scores transposed,
            softmax denominator via the ones column of vt).  The normalize
            chain is deferred to the next head so gpsimd's
            partition_broadcast never sits ahead of the next head's masks
            in the FIFO."""
            hc, hr = h // 2, (h % 2) * 64
            ctp = pq.tile([128, QHW], F32, tag="ctp", bufs=2,
                          name=f"ctp{h}_{qh}")
            kmax = 8 * (qh + 1)
            for t in range(kmax):
                lo = max(0, t * 128 - qh * QHW)   # first visible local col
                chp = lo // 512                    # first contributing chunk
                sp = pq.tile([128, QHW], F32, tag="work", bufs=2,
                             name=f"sp{h}_{qh}_{t}")
                for ch in range(chp, 2):
                    nc.tensor.matmul(
                        sp[:, ch * 512:(ch + 1) * 512],
                        lhsT=kt[hr:hr + 64, hc, t * 128:(t + 1) * 128],
                        rhs=qt[hr:hr + 64, hc,
                               qh * QHW + ch * 512:qh * QHW + (ch + 1) * 512],
                        start=True, stop=True)
                pb = wp.tile([128, QHW], BF16, tag="pb", bufs=3,
                             name=f"pb{h}_{qh}_{t}")
                nc.scalar.activation(pb[:, lo:QHW], sp[:, lo:QHW], AF.Exp)
                if lo > chp * 512:
                    nc.gpsimd.memset(pb[:, chp * 512:lo], 0.0)
                if t >= 8 * qh:   # diagonal tile: causal-mask the block
                    nc.gpsimd.affine_select(
                        out=pb[:, lo:lo + 128], in_=pb[:, lo:lo + 128],
                        compare_op=mybir.AluOpType.is_ge,
                        fill=0.0, base=0, pattern=[[1, 128]],
                        channel_multiplier=-1)
                for ch in range(chp, 2):
                    nc.tensor.matmul(
                        ctp[:, ch * 512:(ch + 1) * 512],
                        lhsT=vt[:, t, h, :],
                        rhs=pb[:, ch * 512:(ch + 1) * 512],
                        start=(t == 0),
                        stop=(t == 8 * qh + 4 * ch + 3))
            # detach PSUM fast (frees ctp bank for head h+2) + recip now;
            # broadcast + multiply deferred one head.
            cst = wp.tile([128, QHW], F32, tag="cst", bufs=2,
                          name=f"cst{h}_{qh}")
            nc.vector.tensor_copy(cst, ctp)
            rec = wp.tile([1, QHW], F32, tag="rec", bufs=2, name=f"rec{h}_{qh}")
            nc.vector.reciprocal_approx_fast(rec, cst[0:1, :])
            rec16 = wp.tile([1, QHW], BF16, tag="rec16", bufs=2,
                            name=f"rec16_{h}_{qh}")
            nc.vector.tensor_copy(rec16, rec)

            def norm():
                # broadcast rec along partitions on the PE (ones x rec,
                # float32r streams at 1 cyc/row) — gpsimd's
                # partition_broadcast lives in a different Q7 library than
                # affine_select and each switch stalls its queue ~8us.
                bcp = pq.tile([128, QHW], F32, tag="work", bufs=2,
                              name=f"bcp{h}_{qh}")
                for ch in range(2):
                    nc.tensor.matmul(
                        bcp[:, ch * 512:(ch + 1) * 512],
                        lhsT=ones_row,
                        rhs=rec16[:, ch * 512:(ch + 1) * 512],
                        start=True, stop=True)
                nc.vector.tensor_mul(
                    ctxT[hr:hr + 64, hc, qh * QHW:(qh + 1) * QHW],
                    cst[64:VW, :], bcp[64:VW, :])

            pending_norm.append(norm)

        def outproj(t):
            """partial out-projection + store for token tile t."""
            osb = wp.tile([128, D], F16, tag="osb", bufs=3, name=f"osb{t}")
            po = pq.tile([128, 1024], F32, tag="work", bufs=2, name=f"po{t}")
            for n in range(2):
                for x in range(2):
                    nc.tensor.matmul(
                        po[:, n * 512:(n + 1) * 512],
                        lhsT=ctxT[:, x, t * 128:(t + 1) * 128],
                        rhs=wob[:, x, n * 512:(n + 1) * 512],
                        start=(x == 0), stop=(x == 1))
            nc.vector.tensor_copy(osb, po)
            eng = nc.sync if t % 2 == 0 else nc.scalar
            eng.dma_start(out_d[t * 128:(t + 1) * 128, :], osb)

        # ---- prologue: tiles 0-7 + QK col-tile0 for tokens 0-1023.
        # DVE queue order: first cast, then the V/K converts (their SWDGE
        # loads land while cast 0 runs), Q's convert after the next cast,
        # Wo's much later — a convert ahead of the casts would stall every
        # transpose behind a 4th-in-queue weight DMA.
        xh0 = wp.tile([128, 2, D], F16, tag="xh", bufs=2, name="xh0pre")
        nc.vector.tensor_copy(xh0, xs_tiles[0])
        nc.vector.tensor_copy(wvh, wvs)
        nc.vector.tensor_copy(wkh, wks)
        xpair_body(0, xh0)
        nc.vector.tensor_copy(wqh, wqs)
        for xp in (2, 1, 3):
            xpair_chain(xp)
        qk_block(wkh, kt, 0, 0)
        qk_block(wqh, qt, 0, 0)

        # ---- attention q-half 0 (k,v tiles 0-7 only)
        attention(0, 0)
        attention(1, 0)
        flush_norm()            # norm(0,0) — emitted after h1's masks
        qk_block(wkh, kt, 1, 0)
        qk_block(wqh, qt, 1, 0)
        attention(2, 0)
        flush_norm()
        attention(3, 0)
        flush_norm()

        # ---- remaining X tiles + projections (pure-PE block; exp stream
        # from q-half 0 still draining on the scalar engine)
        for xp in (4, 6, 5, 7):
            xpair_chain(xp)
        nc.vector.tensor_copy(wob, wos)
        qk_block(wkh, kt, 0, 1)
        qk_block(wqh, qt, 0, 1)

        # ---- attention q-half 1 with out-projection of finished q rows
        attention(0, 1)
        flush_norm()            # norm(3,0): qh0 ctxT complete
        outproj(0)
        outproj(1)
        attention(1, 1)
        flush_norm()
        outproj(2)
        outproj(3)
        qk_block(wkh, kt, 1, 1)
        qk_block(wqh, qt, 1, 1)
        attention(2, 1)
        flush_norm()
        outproj(4)
        outproj(5)
        attention(3, 1)
        flush_norm()
        flush_norm()
        for t in range(6, TOK_TILES):
            outproj(t)


_PROGRAM = None


def _get_program():
    global _PROGRAM
    if _PROGRAM is None:
        _PROGRAM = _build_program()
    return _PROGRAM


def make_in_maps(X, Wq, Wk, Wv, Wo):
    X = np.asarray(X, dtype=np.float32)
    Wq = np.asarray(Wq, dtype=np.float32)
    Wk = np.asarray(Wk, dtype=np.float32)
    Wv = np.asarray(Wv, dtype=np.float32)
    Wo = np.asarray(Wo, dtype=np.float32)
    in_maps = []
    for core in range(NCORES):
        b, hg = core // 4, core % 4
        cs = slice(hg * CG, (hg + 1) * CG)
        in_maps.append({
            "X": np.ascontiguousarray(X[b]),
            "Wq": np.ascontiguousarray(Wq[:, cs]),
            "Wk": np.ascontiguousarray(Wk[:, cs]),
            "Wv": np.ascontiguousarray(Wv[:, cs]),
            "Wo": np.ascontiguousarray(Wo[cs, :]),
        })
    return in_maps


def combine_outputs(results, bo):
    bo = np.asarray(bo, dtype=np.float32)
    out = np.empty((B, S, D), dtype=np.float32)
    for b in range(B):
        acc = results[b * 4]["out"].astype(np.float32)
        for hg in range(1, 4):
            acc += results[b * 4 + hg]["out"].astype(np.float32)
        out[b] = acc + bo[None, :]
    return out


def run(X, Wq, Wk, Wv, Wo, bo, **spmd_kwargs):
    nc = _get_program()
    in_maps = make_in_maps(X, Wq, Wk, Wv, Wo)
    res = run_bass_kernel_spmd(nc, in_maps, core_ids=list(range(NCORES)),
                               **spmd_kwargs)
    return combine_outputs(res.results, bo), res


def kernel(X, Wq, Wk, Wv, Wo, bo):
    out, _ = run(X, Wq, Wk, Wv, Wo, bo)
    return out


# revision 9
# speedup vs baseline: 1.1054x; 1.1054x over previous
"""Multi-head causal attention (B=2, S=2048, D=1024, H=16) on 8 TRN2 NeuronCores.

Sharding: batch x head-group.  Core i handles batch b = i//4 and head-group
hg = i%4 (4 heads = 256 projection columns).  Each core computes
  Q^T/K^T/V = proj(X_b) for its 256 columns, causal attention for its 4
  heads, and a partial output  ctx_slice @ Wo[256-row slice]  ->
  [2048, 1024] fp16 partial.  Host sums the 4 partials per batch (fp32) and
  adds bo (standard tensor-parallel row-sharded out-projection unshard).

v2 structure (vs the 217us baseline):
  - PE warm-up burst at t=0 + junk-MM sprinkles inside the transpose
    chains: PE-transposes don't count as "busy" for the HAM clock gate, so
    without real matmuls in the window the whole prologue runs at 1.2GHz.
  - X is loaded with 8 DMAs (2 token-tiles each) alternating sync/scalar
    HWDGE rings; weights ride the gpsimd SWDGE ring (V first).
  - Engine-queue ordering matters (all queues are FIFO): casts precede
    weight converts on DVE, Wo's convert is emitted only before outproj,
    and softmax normalization (recip/broadcast/mul) is deferred one head
    so gpsimd's partition_broadcast never blocks the next head's masks.
  - qh-major attention with pure-PE blocks (late X transposes + V/QK
    projections, out-projection of finished q-rows) between heads.
  - causal masks + partial-chunk zeroing on gpsimd (frees DVE).
  - fp16 DRAM output (halves output DMA).
PSUM: tag "work" [128,1024]f32 x2 + tag "ctp" x2 = 8 banks exactly.
"""

import numpy as np

import concourse.bass as bass
import concourse.mybir as mybir
import concourse.tile as tile
from concourse import bacc
from concourse.bass_utils import run_bass_kernel_spmd
from concourse.masks import make_identity

F32 = mybir.dt.float32
F32R = mybir.dt.float32r
F16 = mybir.dt.float16
BF16 = mybir.dt.bfloat16
AF = mybir.ActivationFunctionType

B, S, D = 2, 2048, 1024
H, HD = 16, 64
NCORES = 8
CG = 256            # projection columns per core (4 heads)
TOK_TILES = S // 128   # 16
D_CHUNKS = D // 128    # 8
QH = 2              # q halves
QHW = 1024          # q-half width
VW = 128            # [1 | pad | V(64)] per head in vt; V at 64:128 (base-64 slice)


def _build_program():
    nc = bacc.Bacc("TRN2", target_bir_lowering=False, debug=False)

    x_d = nc.dram_tensor("X", [S, D], F32, kind="ExternalInput").ap()
    wq_d = nc.dram_tensor("Wq", [D, CG], F32, kind="ExternalInput").ap()
    wk_d = nc.dram_tensor("Wk", [D, CG], F32, kind="ExternalInput").ap()
    wv_d = nc.dram_tensor("Wv", [D, CG], F32, kind="ExternalInput").ap()
    wo_d = nc.dram_tensor("Wo", [CG, D], F32, kind="ExternalInput").ap()
    out_d = nc.dram_tensor("out", [S, D], F16, kind="ExternalOutput").ap()

    with tile.TileContext(nc) as tc:
        _emit(nc, tc, x_d, wq_d, wk_d, wv_d, wo_d, out_d)
    nc.compile()
    return nc


def _emit(nc, tc, x_d, wq_d, wk_d, wv_d, wo_d, out_d):
    with (
        tc.sbuf_pool(name="persist", bufs=1) as pp,
        tc.sbuf_pool(name="work", bufs=1) as wp,
        tc.psum_pool(name="ps", bufs=1) as pq,
    ):
        # ---- persistent SBUF tensors
        xt = pp.tile([128, D_CHUNKS, S], F16, name="xt")        # X^T  [dval, dchunk, tok]
        qt = pp.tile([128, 2, S], F16, name="qt")               # Q^T  [col, coltile, tok]
        kt = pp.tile([128, 2, S], F16, name="kt")
        vt = pp.tile([128, TOK_TILES, 4, VW], BF16, name="vt")  # [1|pad|V] per (ktile, head)
        ctxT = pp.tile([128, 2, S], BF16, name="ctxT")
        wqh = pp.tile([128, D_CHUNKS, CG], F16, name="wqh")
        wkh = pp.tile([128, D_CHUNKS, CG], F16, name="wkh")
        wvh = pp.tile([128, D_CHUNKS, CG], F16, name="wvh")
        wob = pp.tile([128, 2, D], BF16, name="wob")
        ident = pp.tile([128, 128], F16, name="ident")
        ones_row = pp.tile([1, 128], BF16, name="ones_row")

        # ---- identity first (feeds warm-up + transposes), then PE warm-up:
        # junk matmuls with no data deps keep the PE HAM window busy from
        # ~0us so the projection phase runs at 2.4GHz, not 1.2.
        make_identity(nc, ident)
        nc.gpsimd.memset(ones_row, 1.0)
        pw = pq.tile([128, 1024], F32, tag="ctp", bufs=2, name="pw")
        for i in range(35):
            nc.tensor.matmul(pw[:, 0:128], lhsT=ident, rhs=ident,
                             start=True, stop=True)

        # ---- DMAs: weights on the gpsimd SWDGE ring (V first — needed
        # earliest), X as 8x 2-tile loads alternating sync/scalar HWDGE.
        # weights lead on the sync ring (V then K), X pairs alternate
        # scalar/sync so pair 0 lands first; Wq rides late on scalar, Wo
        # last on sync.  Two HWDGE rings split the 12MB inbound stream.
        wvs = wp.tile([128, D_CHUNKS, CG], F32, tag="wstage", bufs=2, name="wvs")
        nc.sync.dma_start(wvs, wv_d.rearrange("(dc p) c -> p dc c", p=128))
        wks = wp.tile([128, D_CHUNKS, CG], F32, tag="wstage", bufs=2, name="wks")
        nc.sync.dma_start(wks, wk_d.rearrange("(dc p) c -> p dc c", p=128))

        xs_tiles = []
        for xp in range(8):
            xs = wp.tile([128, 2, D], F32, tag="xs", bufs=4, name=f"xs{xp}")
            eng = nc.scalar if xp % 2 == 0 else nc.sync
            eng.dma_start(
                xs, x_d[xp * 256:(xp + 1) * 256, :].rearrange(
                    "(c p) d -> p c d", p=128))
            xs_tiles.append(xs)

        wqs = wp.tile([128, D_CHUNKS, CG], F32, tag="wstage2", bufs=2, name="wqs")
        nc.scalar.dma_start(wqs, wq_d.rearrange("(dc p) c -> p dc c", p=128))
        wos = wp.tile([128, 2, D], F32, tag="wstage2", bufs=2, name="wos")
        nc.sync.dma_start(wos, wo_d.rearrange("(ct p) n -> p ct n", p=128))

        # V ones columns (GPS; right after the SWDGE issues)
        for h in range(4):
            nc.gpsimd.memset(vt[:, :, h, 0:1], 1.0)

        # ---- helpers ------------------------------------------------------
        def xpair_body(xp, xh):
            """transpose + V-projection for token tiles 2xp, 2xp+1.
            Junk matmuls (HAM keep-warm) target the unused bank 1 of the
            V psum tile — transposes alone don't register as PE activity."""
            for tl in range(2):
                t = 2 * xp + tl
                vps = pq.tile([128, 1024], F32, tag="work", bufs=2,
                              name=f"vps{t}")
                for dp in range(2):
                    wk_ps = pq.tile([128, 1024], F32, tag="work", bufs=2,
                                    name=f"xtp{t}_{dp}")
                    xtp = wk_ps[:, 0:256].bitcast(F16)  # [128, 512] f16 view
                    for dd in range(4):
                        d = dp * 4 + dd
                        nc.tensor.transpose(
                            xtp[:, dd * 128:(dd + 1) * 128],
                            xh[:, tl, d * 128:(d + 1) * 128], ident)
                        if dd % 2 == 1:
                            nc.tensor.matmul(vps[:, 512:576], lhsT=ident,
                                             rhs=ident[:, 0:64],
                                             start=True, stop=True)
                    nc.scalar.copy(
                        xt[:, dp * 4:(dp + 1) * 4, t * 128:(t + 1) * 128],
                        xtp.rearrange("p (dd c) -> p dd c", dd=4))
                # V projection for tile t
                for d in range(D_CHUNKS):
                    nc.tensor.matmul(
                        vps[:, 0:CG],
                        lhsT=xt[:, d, t * 128:(t + 1) * 128],
                        rhs=wvh[:, d, :],
                        start=(d == 0), stop=(d == D_CHUNKS - 1))
                nc.vector.tensor_copy(
                    vt[:, t, :, 64:VW],
                    vps[:, 0:CG].rearrange("p (h c) -> p h c", h=4))

        def xpair_chain(xp):
            xh = wp.tile([128, 2, D], F16, tag="xh", bufs=2, name=f"xh{xp}")
            nc.vector.tensor_copy(xh, xs_tiles[xp])
            xpair_body(xp, xh)

        def qk_block(w_sb, dst, ct, t4pair):
            """Q^T or K^T for col-tile ct, tokens [t4pair*1024, +1024).
            d-outer: one stationary load covers two 512-token chunks."""
            ps = pq.tile([128, 1024], F32, tag="work", bufs=2,
                         name=f"qk{ct}_{t4pair}")
            base = t4pair * 1024
            for d in range(D_CHUNKS):
                for ch in range(2):
                    nc.tensor.matmul(
                        ps[:, ch * 512:(ch + 1) * 512],
                        lhsT=w_sb[:, d, ct * 128:(ct + 1) * 128],
                        rhs=xt[:, d, base + ch * 512: base + (ch + 1) * 512],
                        start=(d == 0), stop=(d == D_CHUNKS - 1))
            nc.scalar.copy(dst[:, ct, base:base + 1024], ps)

        pending_norm = []

        def flush_norm():
            while pending_norm:
                pending_norm.pop(0)()

        def attention(h, qh):
            """Causal attention for head h, q-half qh (scores transposed,
            softmax denominator via the ones column of vt).  The normalize
            chain is deferred to the next head so gpsimd's
            partition_broadcast never sits ahead of the next head's masks
            in the FIFO."""
            hc, hr = h // 2, (h % 2) * 64
            ctp = pq.tile([128, QHW], F32, tag="ctp", bufs=2,
                          name=f"ctp{h}_{qh}")
            kmax = 8 * (qh + 1)
            for t in range(kmax):
                lo = max(0, t * 128 - qh * QHW)   # first visible local col
                chp = lo // 512                    # first contributing chunk
                sp = pq.tile([128, QHW], F32, tag="work", bufs=2,
                             name=f"sp{h}_{qh}_{t}")
                for ch in range(chp, 2):
                    nc.tensor.matmul(
                        sp[:, ch * 512:(ch + 1) * 512],
                        lhsT=kt[hr:hr + 64, hc, t * 128:(t + 1) * 128],
                        rhs=qt[hr:hr + 64, hc,
                               qh * QHW + ch * 512:qh * QHW + (ch + 1) * 512],
                        start=True, stop=True)
                pb = wp.tile([128, QHW], BF16, tag="pb", bufs=3,
                             name=f"pb{h}_{qh}_{t}")
                nc.scalar.activation(pb[:, lo:QHW], sp[:, lo:QHW], AF.Exp)
                if lo > chp * 512:
                    nc.gpsimd.memset(pb[:, chp * 512:lo], 0.0)
                if t >= 8 * qh:   # diagonal tile: causal-mask the block
                    nc.gpsimd.affine_select(
                        out=pb[:, lo:lo + 128], in_=pb[:, lo:lo + 128],
                        compare_op=mybir.AluOpType.is_ge,
                        fill=0.0, base=0, pattern=[[1, 128]],
                        channel_multiplier=-1)
                for ch in range(chp, 2):
                    nc.tensor.matmul(
                        ctp[:, ch * 512:(ch + 1) * 512],
                        lhsT=vt[:, t, h, :],
                        rhs=pb[:, ch * 512:(ch + 1) * 512],
                        start=(t == 0),
                        stop=(t == 8 * qh + 4 * ch + 3))
            # detach PSUM fast (frees ctp bank for head h+2) + recip now;
            # broadcast + multiply deferred one head.
            cst = wp.tile([128, QHW], F32, tag="cst", bufs=2,
                          name=f"cst{h}_{qh}")
            nc.vector.tensor_copy(cst, ctp)
            rec = wp.tile([1, QHW], F32, tag="rec", bufs=2, name=f"rec{h}_{qh}")
            nc.vector.reciprocal_approx_fast(rec, cst[0:1, :])
            rec16 = wp.tile([1, QHW], BF16, tag="rec16", bufs=2,
                            name=f"rec16_{h}_{qh}")
            nc.vector.tensor_copy(rec16, rec)

            def norm():
                # broadcast rec along partitions on the PE (ones x rec,
                # float32r streams at 1 cyc/row) — gpsimd's
                # partition_broadcast lives in a different Q7 library than
                # affine_select and each switch stalls its queue ~8us.
                bcp = pq.tile([128, QHW], F32, tag="work", bufs=2,
                              name=f"bcp{h}_{qh}")
                for ch in range(2):
                    nc.tensor.matmul(
                        bcp[:, ch * 512:(ch + 1) * 512],
                        lhsT=ones_row,
                        rhs=rec16[:, ch * 512:(ch + 1) * 512],
                        start=True, stop=True)
                nc.vector.tensor_mul(
                    ctxT[hr:hr + 64, hc, qh * QHW:(qh + 1) * QHW],
                    cst[64:VW, :], bcp[64:VW, :])

            pending_norm.append(norm)

        def outproj(t):
            """partial out-projection + store for token tile t."""
            osb = wp.tile([128, D], F16, tag="osb", bufs=3, name=f"osb{t}")
            po = pq.tile([128, 1024], F32, tag="work", bufs=2, name=f"po{t}")
            for n in range(2):
                for x in range(2):
                    nc.tensor.matmul(
                        po[:, n * 512:(n + 1) * 512],
                        lhsT=ctxT[:, x, t * 128:(t + 1) * 128],
                        rhs=wob[:, x, n * 512:(n + 1) * 512],
                        start=(x == 0), stop=(x == 1))
            nc.vector.tensor_copy(osb, po)
            eng = nc.sync if t % 2 == 0 else nc.scalar
            eng.dma_start(out_d[t * 128:(t + 1) * 128, :], osb)

        # ---- prologue: tiles 0-7 + QK col-tile0 for tokens 0-1023.
        # DVE queue order: first cast, then the V/K converts (their SWDGE
        # loads land while cast 0 runs), Q's convert after the next cast,
        # Wo's much later — a convert ahead of the casts would stall every
        # transpose behind a 4th-in-queue weight DMA.
        xh0 = wp.tile([128, 2, D], F16, tag="xh", bufs=2, name="xh0pre")
        nc.vector.tensor_copy(xh0, xs_tiles[0])
        nc.vector.tensor_copy(wvh, wvs)
        nc.vector.tensor_copy(wkh, wks)
        xpair_body(0, xh0)
        nc.vector.tensor_copy(wqh, wqs)
        for xp in (2, 1, 3):
            xpair_chain(xp)
        qk_block(wkh, kt, 0, 0)
        qk_block(wqh, qt, 0, 0)

        # ---- attention q-half 0 (k,v tiles 0-7 only)
        attention(0, 0)
        attention(1, 0)
        qk_block(wkh, kt, 1, 0)
        qk_block(wqh, qt, 1, 0)
        attention(2, 0)
        attention(3, 0)
        flush_norm()            # all 4 q-half-0 normalizes; their psum
        # broadcasts slot between phases so the "work" rotation never
        # couples a head's score stream to the previous head's DVE mul.

        # ---- remaining X tiles + projections (pure-PE block; exp stream
        # from q-half 0 still draining on the scalar engine)
        for xp in (4, 6, 5, 7):
            xpair_chain(xp)
        nc.vector.tensor_copy(wob, wos)
        qk_block(wkh, kt, 0, 1)
        qk_block(wqh, qt, 0, 1)

        # ---- attention q-half 1 with out-projection of finished q rows
        attention(0, 1)
        outproj(0)
        outproj(1)
        attention(1, 1)
        outproj(2)
        outproj(3)
        qk_block(wkh, kt, 1, 1)
        qk_block(wqh, qt, 1, 1)
        attention(2, 1)
        outproj(4)
        outproj(5)
        attention(3, 1)
        flush_norm()
        for t in range(6, TOK_TILES):
            outproj(t)


_PROGRAM = None


def _get_program():
    global _PROGRAM
    if _PROGRAM is None:
        _PROGRAM = _build_program()
    return _PROGRAM


def make_in_maps(X, Wq, Wk, Wv, Wo):
    X = np.asarray(X, dtype=np.float32)
    Wq = np.asarray(Wq, dtype=np.float32)
    Wk = np.asarray(Wk, dtype=np.float32)
    Wv = np.asarray(Wv, dtype=np.float32)
    Wo = np.asarray(Wo, dtype=np.float32)
    in_maps = []
    for core in range(NCORES):
        b, hg = core // 4, core % 4
        cs = slice(hg * CG, (hg + 1) * CG)
        in_maps.append({
            "X": np.ascontiguousarray(X[b]),
            "Wq": np.ascontiguousarray(Wq[:, cs]),
            "Wk": np.ascontiguousarray(Wk[:, cs]),
            "Wv": np.ascontiguousarray(Wv[:, cs]),
            "Wo": np.ascontiguousarray(Wo[cs, :]),
        })
    return in_maps


def combine_outputs(results, bo):
    bo = np.asarray(bo, dtype=np.float32)
    out = np.empty((B, S, D), dtype=np.float32)
    for b in range(B):
        acc = results[b * 4]["out"].astype(np.float32)
        for hg in range(1, 4):
            acc += results[b * 4 + hg]["out"].astype(np.float32)
        out[b] = acc + bo[None, :]
    return out


def run(X, Wq, Wk, Wv, Wo, bo, **spmd_kwargs):
    nc = _get_program()
    in_maps = make_in_maps(X, Wq, Wk, Wv, Wo)
    res = run_bass_kernel_spmd(nc, in_maps, core_ids=list(range(NCORES)),
                               **spmd_kwargs)
    return combine_outputs(res.results, bo), res


def kernel(X, Wq, Wk, Wv, Wo, bo):
    out, _ = run(X, Wq, Wk, Wv, Wo, bo)
    return out


# revision 17
# speedup vs baseline: 1.4962x; 1.3535x over previous
"""Multi-head causal attention (B=2, S=2048, D=1024, H=16) on 8 TRN2 NeuronCores.

Sharding: batch x head-group.  Core i handles batch b = i//4 and head-group
hg = i%4 (4 heads = 256 projection columns).  Each core computes
  Q^T/K^T/V = proj(X_b) for its 256 columns, causal attention for its 4
  heads, and a partial output  ctx_slice @ Wo[256-row slice]  ->
  [2048, 1024] fp16 partial.  Host sums the 4 partials per batch (in fp32)
  and adds bo (standard tensor-parallel row-sharded out-projection unshard).

On-core algorithm (per core):
  - X^T tiles built once via PE transposes (fp16).
  - Q^T = Wq^T X^T-route: lhsT=Wq chunk, rhs=X^T  -> Q^T [cols, tok] (fp16)
  - scores computed TRANSPOSED: S^T[k, q] = K @ Q^T via lhsT=K^T slice,
    rhs=Q^T, so softmax's k-reduction lands on the partition axis where the
    PE (ones-column trick) does it for free.
  - softmax without row-max: scores are bounded (|s| < 70 << 88), exp in
    fp32 cannot overflow. P = exp(S^T) in bf16.
  - PV: lhsT = [1 | pad | V] (ones col first, V at 32-aligned rows), rhs =
    P^T -> softmax denominator (PSUM row 0) and ctx~^T (rows 64:128) in one
    accumulation.  Detach PSUM with one copy, then normalize asynchronously:
    fast custom-DVE reciprocal + gpsimd partition_broadcast + DVE multiply.
  - out-proj: lhsT = ctx^T (already transposed!), rhs = Wo rows (bf16).
Pre-softmax chain runs in fp16 (1 cyc/row on PE, 8x finer mantissa than
bf16), post-softmax in bf16.

Perf deltas over the original baseline (217us):
  - PE warm-up burst at t=0: junk matmuls on the identity fill the
    DMA-bound start so the HAM clock gate reaches 8/8 before the real
    projection work instead of 45us in (cold = half clock).
  - X tiles are loaded on BOTH HWDGE rings (even tiles on sync after the
    weights, odd tiles on scalar) — the 12MB inbound stream previously
    serialized on one ring at ~half the per-NC HBM bandwidth.
  - fp16 DRAM output (halves output DMA; host still reduces in fp32).
"""

import numpy as np

import concourse.bass as bass
import concourse.mybir as mybir
import concourse.tile as tile
from concourse import bacc
from concourse.bass_utils import run_bass_kernel_spmd
from concourse.masks import make_identity

F32 = mybir.dt.float32
F16 = mybir.dt.float16
BF16 = mybir.dt.bfloat16
AF = mybir.ActivationFunctionType

B, S, D = 2, 2048, 1024
H, HD = 16, 64
NCORES = 8
CG = 256            # projection columns per core (4 heads)
HG_HEADS = 4        # heads per core
TOK_TILES = S // 128   # 16
D_CHUNKS = D // 128    # 8
QH = 2              # q halves of 1024
QHW = 1024          # q-half width
KT = S // 128       # 16 k tiles


def _build_program():
    nc = bacc.Bacc("TRN2", target_bir_lowering=False, debug=False)

    x_d = nc.dram_tensor("X", [S, D], F32, kind="ExternalInput").ap()
    wq_d = nc.dram_tensor("Wq", [D, CG], F32, kind="ExternalInput").ap()
    wk_d = nc.dram_tensor("Wk", [D, CG], F32, kind="ExternalInput").ap()
    wv_d = nc.dram_tensor("Wv", [D, CG], F32, kind="ExternalInput").ap()
    wo_d = nc.dram_tensor("Wo", [CG, D], F32, kind="ExternalInput").ap()
    out_d = nc.dram_tensor("out", [S, D], F16, kind="ExternalOutput").ap()

    with tile.TileContext(nc) as tc:
        _emit(nc, tc, x_d, wq_d, wk_d, wv_d, wo_d, out_d)
    nc.compile()
    return nc


def _emit(nc, tc, x_d, wq_d, wk_d, wv_d, wo_d, out_d):
    with (
        tc.sbuf_pool(name="persist", bufs=1) as pp,
        tc.sbuf_pool(name="work", bufs=1) as wp,
    ):
        # ---- persistent SBUF tensors
        xt = pp.tile([128, D_CHUNKS, S], F16, name="xt")        # X^T  [dval, dchunk, tok]
        qt = pp.tile([128, 2, S], F16, name="qt")               # Q^T  [col, coltile, tok]
        kt = pp.tile([128, 2, S], F16, name="kt")
        vt = pp.tile([128, TOK_TILES, HG_HEADS * 128], BF16, name="vt")  # [1|pad|V] per head
        ctxT = pp.tile([128, 2, S], BF16, name="ctxT")
        wqh = pp.tile([128, D_CHUNKS, CG], F16, name="wqh")
        wkh = pp.tile([128, D_CHUNKS, CG], F16, name="wkh")
        wvh = pp.tile([128, D_CHUNKS, CG], F16, name="wvh")
        wob = pp.tile([128, 2, D], BF16, name="wob")
        ident = pp.tile([128, 128], F16, name="ident")
        cmask = pp.tile([128, 128], BF16, name="cmask")

        # ---- constants
        make_identity(nc, ident)
        # causal 0/1 mask for the diagonal 128x128 block of S^T[k, q]:
        # keep (1.0) where q >= k i.e. col >= partition.
        nc.gpsimd.memset(cmask, 1.0)
        nc.gpsimd.affine_select(
            out=cmask, in_=cmask, compare_op=mybir.AluOpType.is_ge,
            fill=0.0, base=0, pattern=[[1, 128]], channel_multiplier=-1,
        )
        # ones column of [1 | pad | V] (ones FIRST so the softmax denominator
        # lands in PSUM row 0 — custom-DVE reciprocal needs a partition-0
        # input; V starts at col 64 so ctx rows are 32-aligned)
        ones_cols = vt.rearrange("p t (h c) -> p t h c", h=HG_HEADS)[:, :, :, 0:1]
        nc.gpsimd.memset(ones_cols, 1.0)

        # ---- X loads: odd tiles get the scalar HWDGE ring to themselves
        # (issued ahead of all scalar compute, with enough buffers that no
        # issue waits on a consumer — a waiting DMA-issue would block every
        # xt-evac/exp queued behind it); even tiles ride the sync ring
        # behind the weights.  The 12MB inbound stream needs both rings.
        xs_tiles = [None] * TOK_TILES
        for t in range(1, TOK_TILES, 2):
            xs = wp.tile([128, D], F32, tag="xso", bufs=6, name=f"xs{t}")
            nc.scalar.dma_start(xs, x_d[t * 128:(t + 1) * 128, :])
            xs_tiles[t] = xs

        # ---- load + convert weights (sync ring, ahead of the even tiles)
        for w_dram, w_sb in ((wv_d, wvh), (wk_d, wkh), (wq_d, wqh)):
            wstage = wp.tile([128, D_CHUNKS, CG], F32, tag="wstage", bufs=2)
            nc.sync.dma_start(wstage, w_dram.rearrange("(dc p) c -> p dc c", p=128))
            nc.vector.tensor_copy(w_sb, wstage)

        for t in range(0, TOK_TILES, 2):
            xs = wp.tile([128, D], F32, tag="xse", bufs=4, name=f"xs{t}")
            nc.sync.dma_start(xs, x_d[t * 128:(t + 1) * 128, :])
            xs_tiles[t] = xs

        wostage = wp.tile([128, 2, D], F32, tag="wostage", bufs=1)
        nc.sync.dma_start(wostage, wo_d.rearrange("(ct p) n -> p ct n", p=128))

        with tc.psum_pool(name="pp1", bufs=1) as pq:
            # ---- PE warm-up: the first ~14us are DMA-bound, and
            # PE-transposes don't count as HAM activity, so without these
            # the whole projection phase runs at the cold 1.2GHz clock.
            pwarm = pq.tile([128, 128], F32, tag="pwarm", bufs=1, name="pwarm")
            for i in range(130):
                nc.tensor.matmul(pwarm, lhsT=ident, rhs=ident,
                                 start=True, stop=True)

            # ---- X fp16 convert, PE-transpose into xt
            for t in range(TOK_TILES):
                xh = wp.tile([128, D], F16, tag="xh", bufs=3)
                nc.vector.tensor_copy(xh, xs_tiles[t])
                for dp in range(2):
                    xtp = pq.tile([128, 512], F16, tag="xtp", bufs=2)
                    for dd in range(4):
                        d = dp * 4 + dd
                        nc.tensor.transpose(
                            xtp[:, dd * 128:(dd + 1) * 128],
                            xh[:, d * 128:(d + 1) * 128], ident)
                    nc.scalar.copy(
                        xt[:, dp * 4:(dp + 1) * 4, t * 128:(t + 1) * 128],
                        xtp.rearrange("p (dd c) -> p dd c", dd=4))

            # ---- V  (lhsT = X^T tile, rhs = Wv)
            for t in range(TOK_TILES):
                psv = pq.tile([128, CG], F32, tag="vps", bufs=2)
                for d in range(D_CHUNKS):
                    nc.tensor.matmul(
                        psv,
                        lhsT=xt[:, d, t * 128:(t + 1) * 128],
                        rhs=wvh[:, d, :],
                        start=(d == 0), stop=(d == D_CHUNKS - 1))
                nc.vector.tensor_copy(
                    vt.rearrange("p t (h c) -> p t h c", h=HG_HEADS)[:, t, :, 64:128],
                    psv.rearrange("p (h c) -> p h c", h=HG_HEADS))

            # ---- Q^T, K^T col-tile 0 (heads 0-1)
            for w_sb, dst in ((wqh, qt), (wkh, kt)):
                for t4 in range(4):
                    ps = pq.tile([128, 512], F32, tag="qkps", bufs=2)
                    for d in range(D_CHUNKS):
                        nc.tensor.matmul(
                            ps,
                            lhsT=w_sb[:, d, 0:128],
                            rhs=xt[:, d, t4 * 512:(t4 + 1) * 512],
                            start=(d == 0), stop=(d == D_CHUNKS - 1))
                    nc.scalar.copy(dst[:, 0, t4 * 512:(t4 + 1) * 512], ps)

        # ---- attention (h-outer), QK col-tile 1 interleaved, out-proj tail
        with tc.psum_pool(name="pp2", bufs=1) as pa:

            def attention(h):
                hc, hr = h // 2, (h % 2) * 64   # col-tile, row offset in qt/kt
                for qh in range(QH):
                    ctp = pa.tile([128, QHW], F32, tag="ctp", bufs=2)
                    kmax = 8 * (qh + 1)
                    for t in range(kmax):
                        lo = max(0, t * 128 - qh * QHW)   # first visible local col
                        chp = lo // 512                    # first contributing chunk
                        sp = pa.tile([128, QHW], F32, tag="sp", bufs=2)
                        for ch in range(chp, 2):
                            nc.tensor.matmul(
                                sp[:, ch * 512:(ch + 1) * 512],
                                lhsT=kt[hr:hr + 64, hc, t * 128:(t + 1) * 128],
                                rhs=qt[hr:hr + 64, hc,
                                       qh * QHW + ch * 512:qh * QHW + (ch + 1) * 512],
                                start=True, stop=True)
                        pb = wp.tile([128, QHW], BF16, tag="pb", bufs=3)
                        nc.scalar.activation(pb[:, lo:QHW], sp[:, lo:QHW], AF.Exp)
                        if lo > chp * 512:
                            nc.vector.memset(pb[:, chp * 512:lo], 0.0)
                        if t >= 8 * qh:   # diagonal tile: mask the boundary block
                            nc.vector.tensor_mul(
                                pb[:, lo:lo + 128], pb[:, lo:lo + 128], cmask)
                        for ch in range(chp, 2):
                            nc.tensor.matmul(
                                ctp[:, ch * 512:(ch + 1) * 512],
                                lhsT=vt[:, t, h * 128:(h + 1) * 128],
                                rhs=pb[:, ch * 512:(ch + 1) * 512],
                                start=(t == 0),
                                stop=(t == 8 * qh + 4 * ch + 3))
                    # Detach the PSUM accumulator with ONE fast copy so the
                    # normalization chain never blocks later PV matmuls
                    # (PSUM-held stalls re-throttle the PE clock).
                    cst = wp.tile([128, QHW], F32, tag="cst", bufs=2)
                    nc.vector.tensor_copy(cst, ctp)
                    # normalize: ctx^T = ctx~^T * (1/denom), denom is row 0
                    # (custom-DVE recip needs a partition-0 SBUF input)
                    rec = wp.tile([1, QHW], F32, tag="rec", bufs=2)
                    rscr = wp.tile([1, QHW], F32, tag="rscr", bufs=2)
                    nc.vector.reciprocal_approx_accurate(rec, cst[0:1, :], rscr)
                    bcr = wp.tile([128, QHW], F32, tag="bcr", bufs=2)
                    nc.gpsimd.partition_broadcast(bcr, rec, channels=128)
                    nc.vector.tensor_mul(
                        ctxT[hr:hr + 64, hc, qh * QHW:(qh + 1) * QHW],
                        cst[64:128, :], bcr[64:128, :])

            attention(0)
            attention(1)

            # Q^T, K^T col-tile 1 (heads 2-3) — PE filler while heads 0-1's
            # exp-gated attention runs, keeps the PE clock unthrottled
            for w_sb, dst in ((wqh, qt), (wkh, kt)):
                for t4 in range(4):
                    ps1 = pa.tile([128, 512], F32, tag="sp", bufs=2)
                    for d in range(D_CHUNKS):
                        nc.tensor.matmul(
                            ps1,
                            lhsT=w_sb[:, d, 128:256],
                            rhs=xt[:, d, t4 * 512:(t4 + 1) * 512],
                            start=(d == 0), stop=(d == D_CHUNKS - 1))
                    nc.scalar.copy(dst[:, 1, t4 * 512:(t4 + 1) * 512], ps1)

            attention(2)
            attention(3)

            # ---- out-projection
            nc.vector.tensor_copy(wob, wostage)
            for t in range(TOK_TILES):
                osb = wp.tile([128, D], F16, tag="osb", bufs=3)
                for n in range(2):
                    pso = pa.tile([128, 512], F32, tag="sp", bufs=2)
                    for x in range(2):
                        nc.tensor.matmul(
                            pso,
                            lhsT=ctxT[:, x, t * 128:(t + 1) * 128],
                            rhs=wob[:, x, n * 512:(n + 1) * 512],
                            start=(x == 0), stop=(x == 1))
                    if n == 0:
                        nc.scalar.copy(osb[:, n * 512:(n + 1) * 512], pso)
                    else:
                        nc.vector.tensor_copy(osb[:, n * 512:(n + 1) * 512], pso)
                eng = nc.sync if t % 2 == 0 else nc.scalar
                eng.dma_start(out_d[t * 128:(t + 1) * 128, :], osb)


_PROGRAM = None


def _get_program():
    global _PROGRAM
    if _PROGRAM is None:
        _PROGRAM = _build_program()
    return _PROGRAM


def make_in_maps(X, Wq, Wk, Wv, Wo):
    X = np.asarray(X, dtype=np.float32)
    Wq = np.asarray(Wq, dtype=np.float32)
    Wk = np.asarray(Wk, dtype=np.float32)
    Wv = np.asarray(Wv, dtype=np.float32)
    Wo = np.asarray(Wo, dtype=np.float32)
    in_maps = []
    for core in range(NCORES):
        b, hg = core // 4, core % 4
        cs = slice(hg * CG, (hg + 1) * CG)
        in_maps.append({
            "X": np.ascontiguousarray(X[b]),
            "Wq": np.ascontiguousarray(Wq[:, cs]),
            "Wk": np.ascontiguousarray(Wk[:, cs]),
            "Wv": np.ascontiguousarray(Wv[:, cs]),
            "Wo": np.ascontiguousarray(Wo[cs, :]),
        })
    return in_maps


def combine_outputs(results, bo):
    bo = np.asarray(bo, dtype=np.float32)
    out = np.empty((B, S, D), dtype=np.float32)
    for b in range(B):
        acc = results[b * 4]["out"].astype(np.float32)
        for hg in range(1, 4):
            acc += results[b * 4 + hg]["out"].astype(np.float32)
        out[b] = acc + bo[None, :]
    return out


def run(X, Wq, Wk, Wv, Wo, bo, **spmd_kwargs):
    nc = _get_program()
    in_maps = make_in_maps(X, Wq, Wk, Wv, Wo)
    res = run_bass_kernel_spmd(nc, in_maps, core_ids=list(range(NCORES)),
                               **spmd_kwargs)
    return combine_outputs(res.results, bo), res


def kernel(X, Wq, Wk, Wv, Wo, bo):
    out, _ = run(X, Wq, Wk, Wv, Wo, bo)
    return out
